# revision 3
# baseline (speedup 1.0000x reference)
"""Trainium2 Bass kernel for nn_BoundaryGreenBranch.

Math (reference):
    bf = relu(relu(bi @ W0 + b0) @ W1 + b1)            # (B, NBC, HID) tiny
    a  = bf @ G0w[:HID] + G0b                          # (B, NBC, HID) tiny
    c  = coords @ G0w[HID:]                            # (B, NINT, HID) small
    h1 = relu(a[:,:,None,:] + c[:,None,:,:])           # (B, NBC, NINT, HID) huge
    h2 = relu(h1 @ G1w + G1b)                          # huge
    u  = (h2 @ G2w + G2b).sum(bc) / NBC                # (B, NINT, 1)

Sharding: 8 cores = 4 batches x 2 halves of NBC (64 bc each). Host does the
tiny encoder stages; each core does its 64bc x 4096int x 64hid block fully
on-chip; host sums the two partial u's per batch (the bc all-reduce).

On-core schedule (v2a), per quad of 4 bc (pairs packed 2-up on 128 parts):
    pass1 (DVE, 4x fp16 mode): h1 [128, 4096] = relu(cT_dup + a') via dual-op
        tensor_scalar, 2 FD=4096 ops emitted one quad ahead.
    G1 (PE): 4 concurrent quadrant matmuls per 512-int chunk (K=M=64); h2pre
        lands in ONE persistent 6-bank PSUM tile zbig with 3 rotating
        1024-col slots (slot = chunk % 3, subtile deps).
    pass2 evac (the bottleneck): PSUM fp32 -> SBUF fp16 relu+bias. Both
        elementwise engines run it at 1 elem/cycle/lane, so per-instruction
        overhead (ACT ~352cyc, DVE ~190cyc) is what's tunable: adjacent-slot
        chunk PAIRS evac as one FD=2048 op; wrap pairs (slot 2->0) as two
        FD=1024 singles. A build-time greedy balances ACT vs DVE given DVE
        also owns pass1 (~2.6us/quad at 4x).
    G2 (PE): lhsT=[G2w;G2w] [128,1] matmuls accumulate the bc-sum over all
        quads in 2 PSUM u tiles (4 col groups concurrent), lagged a quad.
"""

import numpy as np

B, NBC, HID = 4, 128, 64
NINT = 4096
NCORES = 8
NQUAD = 16  # quads of 4 bc per core (64 bc / 4)
NCH = 8  # interior chunks per quad
CHW = 512  # chunk width per bc-pair tile
SLOTW = 1024  # PSUM slot cols per chunk (2 bc-pairs x 512)
NSLOT = 3

_PROG = {}


def _build_program():
    import concourse.bacc as bacc
    import concourse.tile as tile
    from concourse import mybir

    f32 = mybir.dt.float32
    f16 = mybir.dt.float16
    Relu = mybir.ActivationFunctionType.Relu
    add = mybir.AluOpType.add
    mx = mybir.AluOpType.max

    # measured per-op ns for the greedy engine balance
    T_ACT_PAIR, T_ACT_ONE = 1967.0, 1147.0
    T_DVE_PAIR, T_DVE_ONE = 2349.0, 1274.0
    T_PASS1 = 1282.0

    nc = bacc.Bacc("TRN2")
    d_ct = nc.declare_dram_parameter("ctdup", [128, NINT], f16, isOutput=False)
    d_ap = nc.declare_dram_parameter("apairs", [128, 32], f32, isOutput=False)
    d_g1w = nc.declare_dram_parameter("g1w", [128, HID], f16, isOutput=False)
    d_g2w = nc.declare_dram_parameter("g2w", [128, 1], f16, isOutput=False)
    d_g1b = nc.declare_dram_parameter("g1b2", [128, 1], f32, isOutput=False)
    d_u = nc.declare_dram_parameter("upart", [NCH, CHW], f32, isOutput=True)

    with tile.TileContext(nc) as tc:
        with (
            tc.tile_pool(name="const", bufs=1) as const,
            tc.tile_pool(name="h1", bufs=3) as h1pool,
            tc.tile_pool(name="h2p", bufs=6) as h2ppool,
            tc.tile_pool(name="h2s", bufs=6) as h2spool,
            tc.tile_pool(name="psz", bufs=1, space="PSUM") as pszpool,
            tc.tile_pool(name="psu", bufs=1, space="PSUM") as upool,
            tc.tile_pool(name="outp", bufs=1) as outpool,
        ):
            sb_ap = const.tile([128, 32], f32)
            nc.sync.dma_start(out=sb_ap[:], in_=d_ap[:])
            sb_g1w = const.tile([128, HID], f16)
            nc.sync.dma_start(out=sb_g1w[:], in_=d_g1w[:])
            sb_g2w = const.tile([128, 1], f16)
            nc.sync.dma_start(out=sb_g2w[:], in_=d_g2w[:])
            sb_g1b = const.tile([128, 1], f32)
            nc.sync.dma_start(out=sb_g1b[:], in_=d_g1b[:])

            # warm the ACT Relu table while the big cT DMA runs
            dummy = const.tile([128, 1], f32)
            nc.scalar.activation(out=dummy[:], in_=sb_g1b[:], func=Relu)

            # 8 parallel DMA queues; first 4 cover pass-1's first half
            sb_ct = const.tile([128, NINT], f16)
            for i in range(4):
                qs = slice(i * NINT // 4, (i + 1) * NINT // 4)
                nc.gpsimd.dma_start(out=sb_ct[:, qs], in_=d_ct[:, qs])

            zbig = pszpool.tile([128, NSLOT * SLOTW], f32, name="zbig", tag="zbig")
            psu = [
                upool.tile([128, CHW], f32, name=f"u{i}", tag=f"u{i}")
                for i in range(2)
            ]

            def emit_g1_chunk(h1a, h1b, c, slot):
                sl = slice(c * CHW, (c + 1) * CHW)
                z0 = SLOTW * slot
                nc.tensor.matmul(
                    zbig[0:64, z0 : z0 + CHW], sb_g1w[0:64, :], h1a[0:64, sl],
                    start=True, stop=True, tile_position=(0, 0),
                )
                nc.tensor.matmul(
                    zbig[64:128, z0 : z0 + CHW], sb_g1w[64:128, :], h1a[64:128, sl],
                    start=True, stop=True, tile_position=(64, 64),
                )
                nc.tensor.matmul(
                    zbig[64:128, z0 + CHW : z0 + 2 * CHW], sb_g1w[0:64, :],
                    h1b[0:64, sl],
                    start=True, stop=True, tile_position=(0, 64),
                )
                nc.tensor.matmul(
                    zbig[0:64, z0 + CHW : z0 + 2 * CHW], sb_g1w[64:128, :],
                    h1b[64:128, sl],
                    start=True, stop=True, tile_position=(64, 0),
                )

            h2map = {}  # global chunk -> (tile, col offset)
            eng_t = {"A": 0.0, "D": 0.0}  # greedy accumulators

            def evac_one(g, eng):
                z0 = SLOTW * (g % NSLOT)
                h2 = h2spool.tile([128, SLOTW], f16, tag="h2s")
                src = zbig[:, z0 : z0 + SLOTW]
                if eng == "D":
                    nc.vector.tensor_scalar(
                        out=h2[:], in0=src, scalar1=sb_g1b[:], scalar2=0.0,
                        op0=add, op1=mx,
                    )
                else:
                    nc.scalar.activation(
                        out=h2[:], in_=src, func=Relu, bias=sb_g1b[:], scale=1.0,
                    )
                h2map[g] = (h2, 0)

            def evac_pair(g0, eng):
                z0 = SLOTW * (g0 % NSLOT)
                h2 = h2ppool.tile([128, 2 * SLOTW], f16, tag="h2p")
                src = zbig[:, z0 : z0 + 2 * SLOTW]
                if eng == "D":
                    nc.vector.tensor_scalar(
                        out=h2[:], in0=src, scalar1=sb_g1b[:], scalar2=0.0,
                        op0=add, op1=mx,
                    )
                else:
                    nc.scalar.activation(
                        out=h2[:], in_=src, func=Relu, bias=sb_g1b[:], scale=1.0,
                    )
                h2map[g0] = (h2, 0)
                h2map[g0 + 1] = (h2, SLOTW)

            def emit_evacs(g0):
                """Evac chunks (g0, g0+1); greedy ACT/DVE assignment."""
                if g0 % NSLOT != 2:  # slot-adjacent -> one FD=2048 op
                    if eng_t["A"] + T_ACT_PAIR <= eng_t["D"] + T_DVE_PAIR:
                        eng_t["A"] += T_ACT_PAIR
                        evac_pair(g0, "A")
                    else:
                        eng_t["D"] += T_DVE_PAIR
                        evac_pair(g0, "D")
                else:  # slot wrap -> two singles
                    for g in (g0, g0 + 1):
                        if eng_t["A"] + T_ACT_ONE <= eng_t["D"] + T_DVE_ONE:
                            eng_t["A"] += T_ACT_ONE
                            evac_one(g, "A")
                        else:
                            eng_t["D"] += T_DVE_ONE
                            evac_one(g, "D")

            def emit_g2_batch(qprev, cbase):
                """8 G2 matmuls for prev-quad chunks cbase..cbase+3; 4 PE
                column groups run concurrently."""
                ub = psu[cbase // 4]
                g0 = NCH * qprev + cbase
                for half in range(2):
                    for k in range(4):
                        t, off = h2map[g0 + k]
                        o = off + half * CHW
                        j = 32 * k
                        nc.tensor.matmul(
                            ub[j : j + 1, :], sb_g2w[:], t[:, o : o + CHW],
                            start=(qprev == 0 and half == 0),
                            stop=(qprev == NQUAD - 1 and half == 1),
                            tile_position=(0, j),
                        )

            # initial pass1 for quad 0 (quarters, tracking the cT DMA)
            h1a_n = h1pool.tile([128, NINT], f16, name="h1a", tag="h1a")
            h1b_n = h1pool.tile([128, NINT], f16, name="h1b", tag="h1b")
            for qtr in range(4):
                hs = slice(qtr * NINT // 4, (qtr + 1) * NINT // 4)
                for tile_, col in ((h1a_n, 0), (h1b_n, 1)):
                    nc.vector.tensor_scalar(
                        out=tile_[:, hs], in0=sb_ct[:, hs],
                        scalar1=sb_ap[:, col : col + 1], scalar2=0.0,
                        op0=add, op1=mx,
                    )
            eng_t["D"] += 8 * 535.0

            for q in range(NQUAD):
                h1a, h1b = h1a_n, h1b_n
                if q + 1 < NQUAD:
                    h1a_n = h1pool.tile([128, NINT], f16, name="h1a", tag="h1a")
                    h1b_n = h1pool.tile([128, NINT], f16, name="h1b", tag="h1b")
                for c in range(NCH):
                    g = NCH * q + c
                    emit_g1_chunk(h1a, h1b, c, g % NSLOT)
                    if c % 2 == 1:
                        emit_evacs(g - 1)
                    if q + 1 < NQUAD:
                        if c == 1:
                            nc.vector.tensor_scalar(
                                out=h1a_n[:], in0=sb_ct[:],
                                scalar1=sb_ap[:, 2 * q + 2 : 2 * q + 3],
                                scalar2=0.0, op0=add, op1=mx,
                            )
                            eng_t["D"] += T_PASS1
                        elif c == 5:
                            nc.vector.tensor_scalar(
                                out=h1b_n[:], in0=sb_ct[:],
                                scalar1=sb_ap[:, 2 * q + 3 : 2 * q + 4],
                                scalar2=0.0, op0=add, op1=mx,
                            )
                            eng_t["D"] += T_PASS1
                    if q > 0:
                        if c == 1:
                            emit_g2_batch(q - 1, 0)
                        elif c == 5:
                            emit_g2_batch(q - 1, 4)

            emit_g2_batch(NQUAD - 1, 0)
            emit_g2_batch(NQUAD - 1, 4)

            def evac_u(i):
                so = outpool.tile([128, CHW], f32, name=f"so{i}", tag=f"so{i}")
                if i == 0:
                    nc.vector.tensor_copy(out=so[:], in_=psu[i][:])
                else:
                    nc.scalar.copy(out=so[:], in_=psu[i][:])
                for r in range(4):
                    nc.sync.dma_start(
                        out=d_u[4 * i + r : 4 * i + r + 1, :],
                        in_=so[32 * r : 32 * r + 1, :],
                    )

            evac_u(0)
            evac_u(1)

    nc.compile()
    return nc


def _relu(x):
    return np.maximum(x, 0.0)


def _prepare_in_maps(
    boundary_info, interior_coords, W0, b0, W1, b1,
    G0w, G0b, G1w, G1b, G2w, G2b,
):
    f16 = np.float16
    bi = np.asarray(boundary_info, np.float32)
    coords = np.asarray(interior_coords, np.float32)
    W0, b0, W1, b1 = (np.asarray(t, np.float32) for t in (W0, b0, W1, b1))
    G0w, G0b, G1w, G1b, G2w, G2b = (
        np.asarray(t, np.float32) for t in (G0w, G0b, G1w, G1b, G2w, G2b)
    )

    # tiny encoder stages on host
    bf = _relu(bi @ W0 + b0)
    bf = _relu(bf @ W1 + b1)
    a = bf @ G0w[:HID] + G0b  # (B, NBC, HID)
    cint = coords @ G0w[HID:]  # (B, NINT, HID)

    g1w_sb = np.vstack([G1w, G1w]).astype(f16)
    g2w_sb = np.vstack([G2w, G2w]).astype(f16)
    g1b2 = np.concatenate([G1b, G1b]).reshape(128, 1).astype(np.float32)

    in_maps = []
    for core in range(NCORES):
        b, half = divmod(core, 2)
        cT = np.ascontiguousarray(cint[b].T)  # (64, 4096)
        ctdup = np.vstack([cT, cT]).astype(f16)
        asl = a[b, half * 64 : (half + 1) * 64]  # (64 bc, 64 hid)
        apairs = np.ascontiguousarray(asl.reshape(32, 128).T).astype(np.float32)
        in_maps.append(
            {
                "ctdup": ctdup,
                "apairs": apairs,
                "g1w": g1w_sb,
                "g2w": g2w_sb,
                "g1b2": g1b2,
            }
        )
    return in_maps


def _run(in_maps, **kwargs):
    from concourse.bass_utils import run_bass_kernel_spmd

    if "nc" not in _PROG:
        _PROG["nc"] = _build_program()
    return run_bass_kernel_spmd(_PROG["nc"], in_maps, list(range(NCORES)), **kwargs)


def kernel(
    boundary_info, interior_coords, W0, b0, W1, b1,
    G0w, G0b, G1w, G1b, G2w, G2b, interior_h, interior_w,
):
    in_maps = _prepare_in_maps(
        boundary_info, interior_coords, W0, b0, W1, b1,
        G0w, G0b, G1w, G1b, G2w, G2b,
    )
    res = _run(in_maps)

    u = np.zeros((B, NINT), np.float64)
    for core in range(NCORES):
        b = core // 2
        u[b] += res.results[core]["upart"].reshape(NINT).astype(np.float64)
    u = (u / NBC + np.asarray(G2b, np.float32)[0]).astype(np.float32)
    return u.reshape(B, 1, int(interior_h), int(interior_w))


# revision 5
# speedup vs baseline: 1.7105x; 1.7105x over previous
"""Trainium2 Bass kernel for nn_BoundaryGreenBranch.

Math (reference):
    bf = relu(relu(bi @ W0 + b0) @ W1 + b1)            # (B, NBC, HID) tiny
    a  = bf @ G0w[:HID] + G0b                          # (B, NBC, HID) tiny
    c  = coords @ G0w[HID:]                            # (B, NINT, HID) small
    h1 = relu(a[:,:,None,:] + c[:,None,:,:])           # (B, NBC, NINT, HID) huge
    h2 = relu(h1 @ G1w + G1b)                          # huge
    u  = (h2 @ G2w + G2b).sum(bc) / NBC                # (B, NINT, 1)

Sharding: 8 cores = 4 batches x 2 halves of NBC (64 bc each). Host does the
tiny encoder stages; each core does its 64bc x 4096int x 64hid block fully
on-chip; host sums the two partial u's per batch (the bc all-reduce).

On-core layout (per quad of 4 bc, pairs packed 2-up on 128 partitions):
    pass1: h1 tiles [128, 4096] fp16 by DVE tensor_scalar (4x mode):
        relu(cT_dup + a'_pair) with a' as a per-partition scalar; emitted one
        quad ahead as 4 half-tile pieces ordered AFTER the DVE evacs.
    G1: 4 concurrent quadrant matmuls (tile_position) since K=M=64 fills the
        128x128 PE array -> h2pre in PSUM [128, 1024] (2 banks, 3 slots).
    pass2 (the wall; both engines stream PSUM at 1 elem/cycle/lane):
        relu(h2pre + G1b) PSUM->SBUF fp16. DVE takes the EARLY chunks
        {0,2}(+{4} alt quads) so its evacs aren't queued behind pass1;
        ACT (activation bias trick) takes the late chunks so the slots
        needed by the next quad's first chunks free promptly.
    G2: lhsT=[G2w;G2w] [128,1] matmuls accumulate the bc-sum in PSUM u
        tiles; interleaved 2-per-chunk (lagged a quad) so the PE stays
        continuously busy (HAM stays warm) instead of bursty batches.
"""

import numpy as np

B, NBC, HID = 4, 128, 64
NINT = 4096
NCORES = 8
NQUAD = 16  # quads of 4 bc per core (64 bc / 4)
NCH = 8  # interior chunks of 512
CHW = 512  # chunk width

_PROG = {}


def _build_program():
    import concourse.bacc as bacc
    import concourse.tile as tile
    from concourse import mybir

    f32 = mybir.dt.float32
    f16 = mybir.dt.float16
    Relu = mybir.ActivationFunctionType.Relu
    add = mybir.AluOpType.add
    mx = mybir.AluOpType.max

    nc = bacc.Bacc("TRN2")
    d_ct = nc.declare_dram_parameter("ctdup", [128, NINT], f16, isOutput=False)
    d_ap = nc.declare_dram_parameter("apairs", [128, 32], f32, isOutput=False)
    d_g1w = nc.declare_dram_parameter("g1w", [128, HID], f16, isOutput=False)
    d_g2w = nc.declare_dram_parameter("g2w", [128, 1], f16, isOutput=False)
    d_g1b = nc.declare_dram_parameter("g1b2", [128, 1], f32, isOutput=False)
    d_u = nc.declare_dram_parameter("upart", [NCH, CHW], f32, isOutput=True)

    with tile.TileContext(nc) as tc:
        with (
            tc.tile_pool(name="const", bufs=1) as const,
            tc.tile_pool(name="h1", bufs=3) as h1pool,
            tc.tile_pool(name="h2", bufs=16) as h2pool,
            tc.tile_pool(name="ps", bufs=3, space="PSUM") as pspool,
            tc.tile_pool(name="psu", bufs=1, space="PSUM") as upool,
            tc.tile_pool(name="outp", bufs=1) as outpool,
        ):
            sb_ap = const.tile([128, 32], f32)
            nc.sync.dma_start(out=sb_ap[:], in_=d_ap[:])
            sb_g1w = const.tile([128, HID], f16)
            nc.sync.dma_start(out=sb_g1w[:], in_=d_g1w[:])
            sb_g2w = const.tile([128, 1], f16)
            nc.sync.dma_start(out=sb_g2w[:], in_=d_g2w[:])
            sb_g1b = const.tile([128, 1], f32)
            nc.sync.dma_start(out=sb_g1b[:], in_=d_g1b[:])

            # warm the ACT Relu table while the big cT DMA runs
            dummy = const.tile([128, 1], f32)
            nc.scalar.activation(out=dummy[:], in_=sb_g1b[:], func=Relu)

            # 8 parallel DMA queues; first 4 cover pass-1's first half
            sb_ct = const.tile([128, NINT], f16)
            for i in range(4):
                qs = slice(i * NINT // 4, (i + 1) * NINT // 4)
                nc.gpsimd.dma_start(out=sb_ct[:, qs], in_=d_ct[:, qs])

            psu = [
                upool.tile([128, CHW], f32, name=f"u{i}", tag=f"u{i}")
                for i in range(2)
            ]

            def emit_pass1_piece(q, h1a, h1b, piece):
                """One quarter of next-quad pass1: (tile a/b) x (half lo/hi)."""
                t, lo = divmod(piece, 2)
                hs = slice(lo * NINT // 2, (lo + 1) * NINT // 2)
                tile_, col = (h1a, 2 * q) if t == 0 else (h1b, 2 * q + 1)
                nc.vector.tensor_scalar(
                    out=tile_[:, hs], in0=sb_ct[:, hs],
                    scalar1=sb_ap[:, col : col + 1], scalar2=0.0,
                    op0=add, op1=mx,
                )

            def emit_g2_two(q, c, h2s):
                """Both G2 matmuls (2 bc-pair halves) for quad q's chunk c,
                interleaved into the next quad's chunk stream."""
                ub = psu[c // 4]
                j = 32 * (c % 4)
                for half in range(2):
                    sl = slice(half * CHW, (half + 1) * CHW)
                    nc.tensor.matmul(
                        ub[j : j + 1, :], sb_g2w[:], h2s[c][:, sl],
                        start=(q == 0 and half == 0),
                        stop=(q == NQUAD - 1 and half == 1),
                        tile_position=(0, j),
                    )

            h1a_n = h1pool.tile([128, NINT], f16, name="h1a", tag="h1a")
            h1b_n = h1pool.tile([128, NINT], f16, name="h1b", tag="h1b")
            for qtr in range(4):
                hs = slice(qtr * NINT // 4, (qtr + 1) * NINT // 4)
                for tile_, col in ((h1a_n, 0), (h1b_n, 1)):
                    nc.vector.tensor_scalar(
                        out=tile_[:, hs], in0=sb_ct[:, hs],
                        scalar1=sb_ap[:, col : col + 1], scalar2=0.0,
                        op0=add, op1=mx,
                    )

            PIECE_AT = {1: 0, 2: 2, 4: 1, 5: 3}  # chunk -> next-quad p1 piece
            prev = None  # (q, h2s list of 8) of previous quad
            for q in range(NQUAD):
                h1a, h1b = h1a_n, h1b_n
                if q + 1 < NQUAD:
                    h1a_n = h1pool.tile([128, NINT], f16, name="h1a", tag="h1a")
                    h1b_n = h1pool.tile([128, NINT], f16, name="h1b", tag="h1b")
                dve_set = (0, 2) if q % 2 == 0 else (0, 2, 4)
                h2s = []
                for c in range(NCH):
                    sl = slice(c * CHW, (c + 1) * CHW)
                    ps = pspool.tile([128, 2 * CHW], f32, tag="h2pre")
                    nc.tensor.matmul(
                        ps[0:64, 0:CHW], sb_g1w[0:64, :], h1a[0:64, sl],
                        start=True, stop=True, tile_position=(0, 0),
                    )
                    nc.tensor.matmul(
                        ps[64:128, 0:CHW], sb_g1w[64:128, :], h1a[64:128, sl],
                        start=True, stop=True, tile_position=(64, 64),
                    )
                    nc.tensor.matmul(
                        ps[64:128, CHW : 2 * CHW], sb_g1w[0:64, :], h1b[0:64, sl],
                        start=True, stop=True, tile_position=(0, 64),
                    )
                    nc.tensor.matmul(
                        ps[0:64, CHW : 2 * CHW], sb_g1w[64:128, :], h1b[64:128, sl],
                        start=True, stop=True, tile_position=(64, 0),
                    )
                    h2 = h2pool.tile([128, 2 * CHW], f16, tag="h2")
                    if c in dve_set:
                        nc.vector.tensor_scalar(
                            out=h2[:], in0=ps[:],
                            scalar1=sb_g1b[:], scalar2=0.0, op0=add, op1=mx,
                        )
                    else:
                        nc.scalar.activation(
                            out=h2[:], in_=ps[:], func=Relu,
                            bias=sb_g1b[:], scale=1.0,
                        )
                    h2s.append(h2)
                    if q + 1 < NQUAD and c in PIECE_AT:
                        emit_pass1_piece(q + 1, h1a_n, h1b_n, PIECE_AT[c])
                    if prev is not None:
                        emit_g2_two(prev[0], c, prev[1])
                prev = (q, h2s)

            for c in range(NCH):
                emit_g2_two(prev[0], c, prev[1])

            def evac_u(i):
                so = outpool.tile([128, CHW], f32, name=f"so{i}", tag=f"so{i}")
                if i == 0:
                    nc.vector.tensor_copy(out=so[:], in_=psu[i][:])
                else:
                    nc.scalar.copy(out=so[:], in_=psu[i][:])
                for r in range(4):
                    nc.sync.dma_start(
                        out=d_u[4 * i + r : 4 * i + r + 1, :],
                        in_=so[32 * r : 32 * r + 1, :],
                    )

            evac_u(0)
            evac_u(1)

    nc.compile()
    return nc


def _relu(x):
    return np.maximum(x, 0.0)


def _prepare_in_maps(
    boundary_info, interior_coords, W0, b0, W1, b1,
    G0w, G0b, G1w, G1b, G2w, G2b,
):
    f16 = np.float16
    bi = np.asarray(boundary_info, np.float32)
    coords = np.asarray(interior_coords, np.float32)
    W0, b0, W1, b1 = (np.asarray(t, np.float32) for t in (W0, b0, W1, b1))
    G0w, G0b, G1w, G1b, G2w, G2b = (
        np.asarray(t, np.float32) for t in (G0w, G0b, G1w, G1b, G2w, G2b)
    )

    # tiny encoder stages on host
    bf = _relu(bi @ W0 + b0)
    bf = _relu(bf @ W1 + b1)
    a = bf @ G0w[:HID] + G0b  # (B, NBC, HID)
    cint = coords @ G0w[HID:]  # (B, NINT, HID)

    g1w_sb = np.vstack([G1w, G1w]).astype(f16)
    g2w_sb = np.vstack([G2w, G2w]).astype(f16)
    g1b2 = np.concatenate([G1b, G1b]).reshape(128, 1).astype(np.float32)

    in_maps = []
    for core in range(NCORES):
        b, half = divmod(core, 2)
        cT = np.ascontiguousarray(cint[b].T)  # (64, 4096)
        ctdup = np.vstack([cT, cT]).astype(f16)
        asl = a[b, half * 64 : (half + 1) * 64]  # (64 bc, 64 hid)
        apairs = np.ascontiguousarray(asl.reshape(32, 128).T).astype(np.float32)
        in_maps.append(
            {
                "ctdup": ctdup,
                "apairs": apairs,
                "g1w": g1w_sb,
                "g2w": g2w_sb,
                "g1b2": g1b2,
            }
        )
    return in_maps


def _run(in_maps, **kwargs):
    from concourse.bass_utils import run_bass_kernel_spmd

    if "nc" not in _PROG:
        _PROG["nc"] = _build_program()
    return run_bass_kernel_spmd(_PROG["nc"], in_maps, list(range(NCORES)), **kwargs)


def kernel(
    boundary_info, interior_coords, W0, b0, W1, b1,
    G0w, G0b, G1w, G1b, G2w, G2b, interior_h, interior_w,
):
    in_maps = _prepare_in_maps(
        boundary_info, interior_coords, W0, b0, W1, b1,
        G0w, G0b, G1w, G1b, G2w, G2b,
    )
    res = _run(in_maps)

    u = np.zeros((B, NINT), np.float64)
    for core in range(NCORES):
        b = core // 2
        u[b] += res.results[core]["upart"].reshape(NINT).astype(np.float64)
    u = (u / NBC + np.asarray(G2b, np.float32)[0]).astype(np.float32)
    return u.reshape(B, 1, int(interior_h), int(interior_w))


# revision 6
# speedup vs baseline: 1.8565x; 1.0854x over previous
"""Trainium2 Bass kernel for nn_BoundaryGreenBranch.

Math (reference):
    bf = relu(relu(bi @ W0 + b0) @ W1 + b1)            # (B, NBC, HID) tiny
    a  = bf @ G0w[:HID] + G0b                          # (B, NBC, HID) tiny
    c  = coords @ G0w[HID:]                            # (B, NINT, HID) small
    h1 = relu(a[:,:,None,:] + c[:,None,:,:])           # (B, NBC, NINT, HID) huge
    h2 = relu(h1 @ G1w + G1b)                          # huge
    u  = (h2 @ G2w + G2b).sum(bc) / NBC                # (B, NINT, 1)

Sharding: 8 cores = 4 batches x 2 halves of NBC (64 bc each). Host does the
tiny encoder stages; each core does its 64bc x 4096int x 64hid block fully
on-chip; host sums the two partial u's per batch (the bc all-reduce).

On-core layout (per quad of 4 bc, pairs packed 2-up on 128 partitions):
    pass1: h1 tiles [128, 4096] fp16 by DVE tensor_scalar (4x mode):
        relu(cT_dup + a'_pair) with a' as a per-partition scalar; emitted one
        quad ahead as 4 half-tile pieces ordered AFTER the DVE evacs.
    G1: 4 concurrent quadrant matmuls (tile_position) since K=M=64 fills the
        128x128 PE array -> h2pre in PSUM [128, 1024] (2 banks, 3 slots).
    pass2 (the wall; both engines stream PSUM at 1 elem/cycle/lane):
        relu(h2pre + G1b) PSUM->SBUF fp16. DVE takes the EARLY chunks
        {0,2}(+{4} alt quads) so its evacs aren't queued behind pass1;
        ACT (activation bias trick) takes the late chunks so the slots
        needed by the next quad's first chunks free promptly.
    G2: lhsT=[G2w;G2w] [128,1] matmuls accumulate the bc-sum in PSUM u
        tiles; interleaved 2-per-chunk (lagged a quad) so the PE stays
        continuously busy (HAM stays warm) instead of bursty batches.
"""

import numpy as np

B, NBC, HID = 4, 128, 64
NINT = 4096
NCORES = 8
NQUAD = 16  # quads of 4 bc per core (64 bc / 4)
NCH = 8  # interior chunks of 512
CHW = 512  # chunk width

_PROG = {}


def _build_program():
    import concourse.bacc as bacc
    import concourse.tile as tile
    from concourse import mybir

    f32 = mybir.dt.float32
    f16 = mybir.dt.float16
    Relu = mybir.ActivationFunctionType.Relu
    add = mybir.AluOpType.add
    mx = mybir.AluOpType.max

    nc = bacc.Bacc("TRN2")
    d_ct = nc.declare_dram_parameter("ctdup", [128, NINT], f16, isOutput=False)
    d_ap = nc.declare_dram_parameter("apairs", [128, 32], f32, isOutput=False)
    d_g1w = nc.declare_dram_parameter("g1w", [128, HID], f16, isOutput=False)
    d_g2w = nc.declare_dram_parameter("g2w", [128, 1], f16, isOutput=False)
    d_g1b = nc.declare_dram_parameter("g1b2", [128, 1], f32, isOutput=False)
    d_u = nc.declare_dram_parameter("upart", [NCH, CHW], f32, isOutput=True)

    with tile.TileContext(nc) as tc:
        with (
            tc.tile_pool(name="const", bufs=1) as const,
            tc.tile_pool(name="h1", bufs=3) as h1pool,
            tc.tile_pool(name="h2", bufs=16) as h2pool,
            tc.tile_pool(name="ps", bufs=3, space="PSUM") as pspool,
            tc.tile_pool(name="psu", bufs=1, space="PSUM") as upool,
            tc.tile_pool(name="outp", bufs=1) as outpool,
        ):
            sb_ap = const.tile([128, 32], f32)
            nc.sync.dma_start(out=sb_ap[:], in_=d_ap[:])
            sb_g1w = const.tile([128, HID], f16)
            nc.sync.dma_start(out=sb_g1w[:], in_=d_g1w[:])
            sb_g2w = const.tile([128, 1], f16)
            nc.sync.dma_start(out=sb_g2w[:], in_=d_g2w[:])
            sb_g1b = const.tile([128, 1], f32)
            nc.sync.dma_start(out=sb_g1b[:], in_=d_g1b[:])

            # warm the ACT Relu table while the big cT DMA runs
            dummy = const.tile([128, 1], f32)
            nc.scalar.activation(out=dummy[:], in_=sb_g1b[:], func=Relu)

            # 8 parallel DMA queues; first 4 cover pass-1's first half
            sb_ct = const.tile([128, NINT], f16)
            for i in range(4):
                qs = slice(i * NINT // 4, (i + 1) * NINT // 4)
                nc.gpsimd.dma_start(out=sb_ct[:, qs], in_=d_ct[:, qs])

            psu = [
                upool.tile([128, CHW], f32, name=f"u{i}", tag=f"u{i}")
                for i in range(2)
            ]

            def emit_pass1_piece(q, h1a, h1b, piece):
                """One quarter of next-quad pass1: (tile a/b) x (half lo/hi)."""
                t, lo = divmod(piece, 2)
                hs = slice(lo * NINT // 2, (lo + 1) * NINT // 2)
                tile_, col = (h1a, 2 * q) if t == 0 else (h1b, 2 * q + 1)
                nc.vector.tensor_scalar(
                    out=tile_[:, hs], in0=sb_ct[:, hs],
                    scalar1=sb_ap[:, col : col + 1], scalar2=0.0,
                    op0=add, op1=mx,
                )

            def emit_g2_four(q, cbase, half, h2s):
                """4 G2 matmuls (chunks cbase..cbase+3, one bc-pair half)
                for quad q, on 4 distinct PE column groups (concurrent)."""
                ub = psu[cbase // 4]
                sl = slice(half * CHW, (half + 1) * CHW)
                for k in range(4):
                    j = 32 * k
                    nc.tensor.matmul(
                        ub[j : j + 1, :], sb_g2w[:], h2s[cbase + k][:, sl],
                        start=(q == 0 and half == 0),
                        stop=(q == NQUAD - 1 and half == 1),
                        tile_position=(0, j),
                    )

            h1a_n = h1pool.tile([128, NINT], f16, name="h1a", tag="h1a")
            h1b_n = h1pool.tile([128, NINT], f16, name="h1b", tag="h1b")
            for qtr in range(4):
                hs = slice(qtr * NINT // 4, (qtr + 1) * NINT // 4)
                for tile_, col in ((h1a_n, 0), (h1b_n, 1)):
                    nc.vector.tensor_scalar(
                        out=tile_[:, hs], in0=sb_ct[:, hs],
                        scalar1=sb_ap[:, col : col + 1], scalar2=0.0,
                        op0=add, op1=mx,
                    )

            PIECE_AT = {1: 0, 2: 2, 4: 1, 5: 3}  # chunk -> next-quad p1 piece
            prev = None  # (q, h2s list of 8) of previous quad
            for q in range(NQUAD):
                h1a, h1b = h1a_n, h1b_n
                if q + 1 < NQUAD:
                    h1a_n = h1pool.tile([128, NINT], f16, name="h1a", tag="h1a")
                    h1b_n = h1pool.tile([128, NINT], f16, name="h1b", tag="h1b")
                dve_set = (2, 5) if q % 2 == 0 else (2, 5, 7)
                h2s = []
                for c in range(NCH):
                    sl = slice(c * CHW, (c + 1) * CHW)
                    ps = pspool.tile([128, 2 * CHW], f32, tag="h2pre")
                    nc.tensor.matmul(
                        ps[0:64, 0:CHW], sb_g1w[0:64, :], h1a[0:64, sl],
                        start=True, stop=True, tile_position=(0, 0),
                    )
                    nc.tensor.matmul(
                        ps[64:128, 0:CHW], sb_g1w[64:128, :], h1a[64:128, sl],
                        start=True, stop=True, tile_position=(64, 64),
                    )
                    nc.tensor.matmul(
                        ps[64:128, CHW : 2 * CHW], sb_g1w[0:64, :], h1b[0:64, sl],
                        start=True, stop=True, tile_position=(0, 64),
                    )
                    nc.tensor.matmul(
                        ps[0:64, CHW : 2 * CHW], sb_g1w[64:128, :], h1b[64:128, sl],
                        start=True, stop=True, tile_position=(64, 0),
                    )
                    h2 = h2pool.tile([128, 2 * CHW], f16, tag="h2")
                    if c in dve_set:
                        nc.vector.tensor_scalar(
                            out=h2[:], in0=ps[:],
                            scalar1=sb_g1b[:], scalar2=0.0, op0=add, op1=mx,
                        )
                    else:
                        nc.scalar.activation(
                            out=h2[:], in_=ps[:], func=Relu,
                            bias=sb_g1b[:], scale=1.0,
                        )
                    h2s.append(h2)
                    if q + 1 < NQUAD and c in PIECE_AT:
                        emit_pass1_piece(q + 1, h1a_n, h1b_n, PIECE_AT[c])
                    if prev is not None and c in (1, 3, 5, 7):
                        emit_g2_four(prev[0], 4 * (c // 4), (c % 4) // 2, prev[1])
                prev = (q, h2s)

            for c in (1, 3, 5, 7):
                emit_g2_four(prev[0], 4 * (c // 4), (c % 4) // 2, prev[1])

            def evac_u(i):
                so = outpool.tile([128, CHW], f32, name=f"so{i}", tag=f"so{i}")
                if i == 0:
                    nc.vector.tensor_copy(out=so[:], in_=psu[i][:])
                else:
                    nc.scalar.copy(out=so[:], in_=psu[i][:])
                for r in range(4):
                    nc.sync.dma_start(
                        out=d_u[4 * i + r : 4 * i + r + 1, :],
                        in_=so[32 * r : 32 * r + 1, :],
                    )

            evac_u(0)
            evac_u(1)

    nc.compile()
    return nc


def _relu(x):
    return np.maximum(x, 0.0)


def _prepare_in_maps(
    boundary_info, interior_coords, W0, b0, W1, b1,
    G0w, G0b, G1w, G1b, G2w, G2b,
):
    f16 = np.float16
    bi = np.asarray(boundary_info, np.float32)
    coords = np.asarray(interior_coords, np.float32)
    W0, b0, W1, b1 = (np.asarray(t, np.float32) for t in (W0, b0, W1, b1))
    G0w, G0b, G1w, G1b, G2w, G2b = (
        np.asarray(t, np.float32) for t in (G0w, G0b, G1w, G1b, G2w, G2b)
    )

    # tiny encoder stages on host
    bf = _relu(bi @ W0 + b0)
    bf = _relu(bf @ W1 + b1)
    a = bf @ G0w[:HID] + G0b  # (B, NBC, HID)
    cint = coords @ G0w[HID:]  # (B, NINT, HID)

    g1w_sb = np.vstack([G1w, G1w]).astype(f16)
    g2w_sb = np.vstack([G2w, G2w]).astype(f16)
    g1b2 = np.concatenate([G1b, G1b]).reshape(128, 1).astype(np.float32)

    in_maps = []
    for core in range(NCORES):
        b, half = divmod(core, 2)
        cT = np.ascontiguousarray(cint[b].T)  # (64, 4096)
        ctdup = np.vstack([cT, cT]).astype(f16)
        asl = a[b, half * 64 : (half + 1) * 64]  # (64 bc, 64 hid)
        apairs = np.ascontiguousarray(asl.reshape(32, 128).T).astype(np.float32)
        in_maps.append(
            {
                "ctdup": ctdup,
                "apairs": apairs,
                "g1w": g1w_sb,
                "g2w": g2w_sb,
                "g1b2": g1b2,
            }
        )
    return in_maps


def _run(in_maps, **kwargs):
    from concourse.bass_utils import run_bass_kernel_spmd

    if "nc" not in _PROG:
        _PROG["nc"] = _build_program()
    return run_bass_kernel_spmd(_PROG["nc"], in_maps, list(range(NCORES)), **kwargs)


def kernel(
    boundary_info, interior_coords, W0, b0, W1, b1,
    G0w, G0b, G1w, G1b, G2w, G2b, interior_h, interior_w,
):
    in_maps = _prepare_in_maps(
        boundary_info, interior_coords, W0, b0, W1, b1,
        G0w, G0b, G1w, G1b, G2w, G2b,
    )
    res = _run(in_maps)

    u = np.zeros((B, NINT), np.float64)
    for core in range(NCORES):
        b = core // 2
        u[b] += res.results[core]["upart"].reshape(NINT).astype(np.float64)
    u = (u / NBC + np.asarray(G2b, np.float32)[0]).astype(np.float32)
    return u.reshape(B, 1, int(interior_h), int(interior_w))
